# revision 22
# baseline (speedup 1.0000x reference)
"""Trainium2 Bass kernel for a 4-layer LSTM cell stack + final FC.

Strategy: data-parallel over batch across 8 NeuronCores (512 rows/core,
weights replicated, no collectives). Per core, per layer:
  gates[512, 4096] = x_in @ Wi[l] + h_prev[l] @ Wh[l] + (bi[l] + bh[l])
computed as PSUM-accumulated fp32r matmuls (batch on partitions), with a
rank-1 ones-matmul adding the bias row. ScalarE applies sigmoid/tanh while
evacuating PSUM; VectorE does the c/h elementwise updates. Activations are
transposed on-chip with PE-transpose to serve as the stationary operand of
the next layer's matmuls.
"""

import sys

sys.path.insert(0, "/opt/trn_rl_repo")

import numpy as np

import concourse.bass as bass
import concourse.tile as tile
from concourse import bacc, mybir
from concourse.bass_utils import run_bass_kernel_spmd
from concourse.masks import make_identity

L, B, IN, H, OUT = 4, 4096, 1024, 1024, 1024
NCORES = 8
BC = B // NCORES  # 512 batch rows per core
P = 128
MT = BC // P  # 4 batch tiles
KT = IN // P  # 8 contraction tiles per 1024
NBS = 512  # gate-column block (PSUM bank width in fp32)
HF = H // NBS  # 2 h-column halves
FC_NB = OUT // NBS

f32 = mybir.dt.float32
f32r = mybir.dt.float32r

Sigmoid = mybir.ActivationFunctionType.Sigmoid
Tanh = mybir.ActivationFunctionType.Tanh

_cache = {}


def _emit(nc):
    x_d = nc.dram_tensor("x", [BC, IN], f32, kind="ExternalInput").ap()
    hp_d = nc.dram_tensor("h_prev", [L, BC, H], f32, kind="ExternalInput").ap()
    cp_d = nc.dram_tensor("c_prev", [L, BC, H], f32, kind="ExternalInput").ap()
    wi_d = nc.dram_tensor("Wi", [L, IN, 4 * H], f32r, kind="ExternalInput").ap()
    wh_d = nc.dram_tensor("Wh", [L, H, 4 * H], f32r, kind="ExternalInput").ap()
    bias_d = nc.dram_tensor("bias", [L, 4 * H], f32, kind="ExternalInput").ap()
    fcw_d = nc.dram_tensor("fc_W", [H, OUT], f32r, kind="ExternalInput").ap()
    fcb_d = nc.dram_tensor("fc_b", [1, OUT], f32, kind="ExternalInput").ap()
    y_d = nc.dram_tensor("y", [BC, OUT], f32, kind="ExternalOutput").ap()
    ho_d = nc.dram_tensor("h_out", [L, BC, H], f32, kind="ExternalOutput").ap()
    co_d = nc.dram_tensor("c_out", [L, BC, H], f32, kind="ExternalOutput").ap()

    with tile.TileContext(nc) as tc:
        _build(nc, tc, x_d, hp_d, cp_d, wi_d, wh_d, bias_d, fcw_d, fcb_d, y_d, ho_d, co_d)


def _build(nc, tc, x_d, hp_d, cp_d, wi_d, wh_d, bias_d, fcw_d, fcb_d, y_d, ho_d, co_d):
    from contextlib import ExitStack

    with ExitStack() as ctx:
        const_pool = ctx.enter_context(tc.tile_pool(name="const", bufs=1))
        wpool = ctx.enter_context(tc.tile_pool(name="weights", bufs=4))
        statT = ctx.enter_context(tc.tile_pool(name="statT", bufs=34))
        gates_pool = ctx.enter_context(tc.tile_pool(name="gates", bufs=16))
        cwork = ctx.enter_context(tc.tile_pool(name="cwork", bufs=6))
        ld_pool = ctx.enter_context(tc.tile_pool(name="ld", bufs=2))
        bias_pool = ctx.enter_context(tc.tile_pool(name="biasp", bufs=2))
        bcast_pool = ctx.enter_context(tc.tile_pool(name="bcast", bufs=2))
        gpsum = ctx.enter_context(tc.tile_pool(name="gpsum", bufs=6, space="PSUM"))
        tpsum = ctx.enter_context(tc.tile_pool(name="tpsum", bufs=2, space="PSUM"))

        ident = const_pool.tile([P, P], f32)
        make_identity(nc, ident)

        def bcast_rows(rows):
            """rows: list of [1, NBS] DRAM APs. Returns [128, len(rows), NBS]
            tile with each row replicated to all partitions via log2-doubling
            SBUF->SBUF DMAs (no PE/DVE cost)."""
            nch = len(rows)
            bc = bcast_pool.tile([P, nch, NBS], f32, tag="bc", name="bc")
            for q, src in enumerate(rows):
                nc.sync.dma_start(bc[0:1, q, :], src)
            p = 1
            while p < P:
                nc.sync.dma_start(bc[p : 2 * p, :, :], bc[0:p, :, :])
                p *= 2
            return bc

        def load_bias_bcast(l, n_list):
            return bcast_rows(
                [bias_d[l : l + 1, n * NBS : (n + 1) * NBS] for n in n_list]
            )

    # --- helpers -----------------------------------------------------------
        def transpose_block(src_128x128_f32, dst_f32r_slice):
            ps = tpsum.tile([P, P], f32, tag="tps")
            nc.tensor.transpose(ps[:], src_128x128_f32, ident[:])
            nc.scalar.copy(dst_f32r_slice, ps[:])

        def load_transposed(src_dram_2d):
            """[BC, 1024] DRAM f32 -> list of KT stationary tiles [128, BC] f32r."""
            kt_tiles = [
                statT.tile([P, BC], f32r, tag="sT", name=f"sT{k}") for k in range(KT)
            ]
            for m in range(MT):
                ld = ld_pool.tile([P, IN], f32, tag="ld")
                nc.sync.dma_start(ld[:], src_dram_2d[m * P : (m + 1) * P, :])
                for k in range(KT):
                    transpose_block(
                        ld[:, k * P : (k + 1) * P], kt_tiles[k][:, m * P : (m + 1) * P]
                    )
            return kt_tiles

        def load_weight_block(w_dram_2d, n):
            """[1024, 4H] f32r DRAM, gate-column block n -> tile [128, KT, 512]."""
            t = wpool.tile([P, KT, NBS], f32r, tag="w")
            src = w_dram_2d.rearrange("(kt p) n -> p kt n", p=P)
            nc.sync.dma_start(t[:], src[:, :, n * NBS : (n + 1) * NBS])
            return t

    # --- phase helpers -----------------------------------------------------
        def phase_A(l, n, hpT_list):
            """h_prev @ Wh part: 8 k-matmuls per m into fresh PSUM groups.
            Independent of the previous layer's h -> keeps PE fed (and the
            HAM clock warm) across half-layer boundaries."""
            wh_t = load_weight_block(wh_d[l], n)
            ps_list = []
            for m in range(MT):
                ps = gpsum.tile([P, NBS], f32, tag="gps", name=f"ps{l}_{n}_{m}")
                for k in range(KT):
                    nc.tensor.matmul(
                        ps[:],
                        hpT_list[k][:, m * P : (m + 1) * P],
                        wh_t[:, k, :],
                        start=(k == 0),
                        stop=False,
                    )
                ps_list.append(ps)
            return ps_list

        def phase_B(l, n, q, xT_list, ps_list, bc):
            """x @ Wi part + bias (DVE) + activated PSUM evacuation."""
            wi_t = load_weight_block(wi_d[l], n)
            out = []
            for m in range(MT):
                ps = ps_list[m]
                for k in range(KT):
                    nc.tensor.matmul(
                        ps[:],
                        xT_list[k][:, m * P : (m + 1) * P],
                        wi_t[:, k, :],
                        start=False,
                        stop=(k == KT - 1),
                    )
                nc.vector.tensor_add(ps[:], ps[:], bc[:, q, :])
                g = gates_pool.tile([P, NBS], f32, tag="g", name=f"g{l}_{n}_{m}")
                nc.scalar.activation(g[:], ps[:], Tanh if q == 2 else Sigmoid)
                out.append(g)
            return out

        def fc_phase_A(n, h3T):
            """First-half fc matmuls (k 0..3) — hoistable across the last
            layer boundary since k 0..3 of h3^T are ready after hf=0."""
            wf_t = load_weight_block(fcw_d, n)
            ps_list = []
            for m in range(MT):
                ps = gpsum.tile([P, NBS], f32, tag="gps", name=f"psfc{n}_{m}")
                for k in range(KT // 2):
                    nc.tensor.matmul(
                        ps[:],
                        h3T[k][:, m * P : (m + 1) * P],
                        wf_t[:, k, :],
                        start=(k == 0),
                        stop=False,
                    )
                ps_list.append(ps)
            return ps_list, wf_t

        def emit_updates(l, hf, gate_sb, xT_next):
            for m in range(MT):
                i_t = gate_sb[(0, m)]
                f_t = gate_sb[(1, m)]
                g_t = gate_sb[(2, m)]
                o_t = gate_sb[(3, m)]
                cp = cwork.tile([P, NBS], f32, tag="cw", name="cp")
                nc.sync.dma_start(
                    cp[:],
                    cp_d[l][m * P : (m + 1) * P, hf * NBS : (hf + 1) * NBS],
                )
                nc.vector.tensor_mul(f_t[:], f_t[:], cp[:])  # f*c
                nc.vector.tensor_mul(i_t[:], i_t[:], g_t[:])  # i*g
                cnew = cwork.tile([P, NBS], f32, tag="cw", name="cnew")
                nc.vector.tensor_add(cnew[:], f_t[:], i_t[:])
                nc.sync.dma_start(
                    co_d[l][m * P : (m + 1) * P, hf * NBS : (hf + 1) * NBS],
                    cnew[:],
                )
                th = cwork.tile([P, NBS], f32, tag="cw", name="th")
                nc.scalar.activation(th[:], cnew[:], Tanh)
                h_t = cwork.tile([P, NBS], f32, tag="cw", name="h_t")
                nc.vector.tensor_mul(h_t[:], o_t[:], th[:])
                nc.sync.dma_start(
                    ho_d[l][m * P : (m + 1) * P, hf * NBS : (hf + 1) * NBS],
                    h_t[:],
                )
                # transpose h half-block into next layer's stationary tiles
                for kk in range(NBS // P):
                    kg = hf * (NBS // P) + kk
                    transpose_block(
                        h_t[:, kk * P : (kk + 1) * P],
                        xT_next[kg][:, m * P : (m + 1) * P],
                    )

        def prep_hprevT_mblock(l, hpT_list, m):
            ld = ld_pool.tile([P, IN], f32, tag="ld", name="hpld")
            nc.sync.dma_start(ld[:], hp_d[l][m * P : (m + 1) * P, :])
            for k in range(KT):
                transpose_block(
                    ld[:, k * P : (k + 1) * P], hpT_list[k][:, m * P : (m + 1) * P]
                )

    # --- prologue: x^T and h_prev[0]^T -------------------------------------
        xT = load_transposed(x_d)
        hpT = load_transposed(hp_d[0])

    # --- layers ------------------------------------------------------------
        pending_A = None
        for l in range(L):
            xT_next = [
                statT.tile([P, BC], f32r, tag="sT", name=f"xTn{l}_{k}")
                for k in range(KT)
            ]
            hpT_next = (
                [
                    statT.tile([P, BC], f32r, tag="sT", name=f"hpTn{l}_{k}")
                    for k in range(KT)
                ]
                if l + 1 < L
                else None
            )
            for hf in range(HF):
                n_list = [2 * q + hf for q in range(4)]
                bc = load_bias_bcast(l, n_list)
                gate_sb = {}
                for q, n in enumerate(n_list):
                    if q == 0 and pending_A is not None:
                        ps_list = pending_A
                        pending_A = None
                    else:
                        ps_list = phase_A(l, n, hpT)
                    gs = phase_B(l, n, q, xT, ps_list, bc)
                    for m in range(MT):
                        gate_sb[(q, m)] = gs[m]
                    # spread next layer's h_prev^T prep across hf=0 n-blocks
                    if hf == 0 and hpT_next is not None:
                        prep_hprevT_mblock(l + 1, hpT_next, q)
                # hoist the next half's h_prev matmuls ahead of the
                # elementwise epilogue so PE never idles at the boundary
                if hf == 0:
                    pending_A = phase_A(l, 1, hpT)
                elif l + 1 < L:
                    pending_A = phase_A(l + 1, 0, hpT_next)
                elif l == L - 1:
                    # last boundary: hoist the first-half fc matmuls (k 0..3
                    # use h3^T tiles produced by the hf=0 updates)
                    pending_A = fc_phase_A(0, xT_next)
                emit_updates(l, hf, gate_sb, xT_next)

            xT = xT_next
            hpT = hpT_next

    # --- final fc: y = h3 @ fc_W + fc_b ------------------------------------
        fcb_bc = bcast_rows(
            [fcb_d[:, n * NBS : (n + 1) * NBS] for n in range(FC_NB)]
        )
        for n in range(FC_NB):
            if n == 0 and pending_A is not None:
                ps_list, wf_t = pending_A
                pending_A = None
            else:
                ps_list, wf_t = fc_phase_A(n, xT)
            for m in range(MT):
                ps = ps_list[m]
                for k in range(KT // 2, KT):
                    nc.tensor.matmul(
                        ps[:],
                        xT[k][:, m * P : (m + 1) * P],
                        wf_t[:, k, :],
                        start=False,
                        stop=(k == KT - 1),
                    )
                nc.vector.tensor_add(ps[:], ps[:], fcb_bc[:, n, :])
                y_sb = gates_pool.tile([P, NBS], f32, tag="g", name=f"y{n}_{m}")
                nc.scalar.copy(y_sb[:], ps[:])
                nc.sync.dma_start(
                    y_d[m * P : (m + 1) * P, n * NBS : (n + 1) * NBS], y_sb[:]
                )


def _build_nc():
    nc = bacc.Bacc("TRN2", target_bir_lowering=False, debug=False, num_devices=NCORES)
    _emit(nc)
    nc.compile()
    return nc


def kernel(x, h_prev, c_prev, Wi, Wh, bi, bh, fc_W, fc_b, **extra):
    nc = _cache.get("nc")
    if nc is None:
        nc = _cache["nc"] = _build_nc()

    bias = np.ascontiguousarray((np.asarray(bi) + np.asarray(bh)).astype(np.float32))
    wi = np.ascontiguousarray(np.asarray(Wi, dtype=np.float32))
    wh = np.ascontiguousarray(np.asarray(Wh, dtype=np.float32))
    fcw = np.ascontiguousarray(np.asarray(fc_W, dtype=np.float32))
    fcb = np.ascontiguousarray(np.asarray(fc_b, dtype=np.float32).reshape(1, OUT))
    x = np.asarray(x, dtype=np.float32)
    h_prev = np.asarray(h_prev, dtype=np.float32)
    c_prev = np.asarray(c_prev, dtype=np.float32)

    in_maps = []
    for i in range(NCORES):
        b0 = i * BC
        in_maps.append(
            {
                "x": np.ascontiguousarray(x[b0 : b0 + BC]),
                "h_prev": np.ascontiguousarray(h_prev[:, b0 : b0 + BC]),
                "c_prev": np.ascontiguousarray(c_prev[:, b0 : b0 + BC]),
                "Wi": wi,
                "Wh": wh,
                "bias": bias,
                "fc_W": fcw,
                "fc_b": fcb,
            }
        )

    res = run_bass_kernel_spmd(nc, in_maps, list(range(NCORES)))
    _cache["last_result"] = res
    y = np.concatenate([res.results[i]["y"] for i in range(NCORES)], axis=0)
    h_out = np.concatenate([res.results[i]["h_out"] for i in range(NCORES)], axis=1)
    c_out = np.concatenate([res.results[i]["c_out"] for i in range(NCORES)], axis=1)
    return y, h_out, c_out


# revision 29
# speedup vs baseline: 1.2158x; 1.2158x over previous
"""Trainium2 Bass kernel for a 4-layer LSTM cell stack + final FC.

Strategy: data-parallel over batch across 8 NeuronCores (512 rows/core,
weights replicated, no collectives). Per core, per layer:
  gates[512, 4096] = x_in @ Wi[l] + h_prev[l] @ Wh[l] + (bi[l] + bh[l])
computed as PSUM-accumulated fp32r matmuls (batch on partitions), with a
rank-1 ones-matmul adding the bias row. ScalarE applies sigmoid/tanh while
evacuating PSUM; VectorE does the c/h elementwise updates. Activations are
transposed on-chip with PE-transpose to serve as the stationary operand of
the next layer's matmuls.
"""

import sys

sys.path.insert(0, "/opt/trn_rl_repo")

import numpy as np

import concourse.bass as bass
import concourse.tile as tile
from concourse import bacc, mybir
from concourse.bass_utils import run_bass_kernel_spmd
from concourse.masks import make_identity

L, B, IN, H, OUT = 4, 4096, 1024, 1024, 1024
NCORES = 8
BC = B // NCORES  # 512 batch rows per core
P = 128
MT = BC // P  # 4 batch tiles
KT = IN // P  # 8 contraction tiles per 1024
NBS = 512  # gate-column block (PSUM bank width in fp32)
HF = H // NBS  # 2 h-column halves
FC_NB = OUT // NBS

f32 = mybir.dt.float32
f32r = mybir.dt.float32r
bf16 = mybir.dt.bfloat16

Sigmoid = mybir.ActivationFunctionType.Sigmoid
Tanh = mybir.ActivationFunctionType.Tanh

_cache = {}


def _emit(nc):
    x_d = nc.dram_tensor("x", [BC, IN], f32, kind="ExternalInput").ap()
    hp_d = nc.dram_tensor("h_prev", [L, BC, H], f32, kind="ExternalInput").ap()
    cp_d = nc.dram_tensor("c_prev", [L, BC, H], f32, kind="ExternalInput").ap()
    wi_d = nc.dram_tensor("Wi", [L, IN, 4 * H], f32, kind="ExternalInput").ap()
    wh_d = nc.dram_tensor("Wh", [L, H, 4 * H], f32, kind="ExternalInput").ap()
    bias_d = nc.dram_tensor("bias", [L, 4 * H], f32, kind="ExternalInput").ap()
    fcw_d = nc.dram_tensor("fc_W", [H, OUT], f32, kind="ExternalInput").ap()
    fcb_d = nc.dram_tensor("fc_b", [1, OUT], f32, kind="ExternalInput").ap()
    y_d = nc.dram_tensor("y", [BC, OUT], f32, kind="ExternalOutput").ap()
    ho_d = nc.dram_tensor("h_out", [L, BC, H], f32, kind="ExternalOutput").ap()
    co_d = nc.dram_tensor("c_out", [L, BC, H], f32, kind="ExternalOutput").ap()

    with tile.TileContext(nc) as tc:
        _build(nc, tc, x_d, hp_d, cp_d, wi_d, wh_d, bias_d, fcw_d, fcb_d, y_d, ho_d, co_d)


def _build(nc, tc, x_d, hp_d, cp_d, wi_d, wh_d, bias_d, fcw_d, fcb_d, y_d, ho_d, co_d):
    from contextlib import ExitStack

    with ExitStack() as ctx:
        const_pool = ctx.enter_context(tc.tile_pool(name="const", bufs=1))
        wpool = ctx.enter_context(tc.tile_pool(name="weights", bufs=6))
        statT = ctx.enter_context(tc.tile_pool(name="statT", bufs=34))
        gates_pool = ctx.enter_context(tc.tile_pool(name="gates", bufs=20))
        cwork = ctx.enter_context(tc.tile_pool(name="cwork", bufs=10))
        ld_pool = ctx.enter_context(tc.tile_pool(name="ld", bufs=4))
        bias_pool = ctx.enter_context(tc.tile_pool(name="biasp", bufs=4))
        gpsum = ctx.enter_context(tc.tile_pool(name="gpsum", bufs=6, space="PSUM"))
        tpsum = ctx.enter_context(tc.tile_pool(name="tpsum", bufs=2, space="PSUM"))

        ident = const_pool.tile([P, P], f32)
        make_identity(nc, ident)
        ones_f32 = const_pool.tile([1, P], f32)
        nc.gpsimd.memset(ones_f32[:], 1.0)
        ones = const_pool.tile([1, P], bf16)
        nc.scalar.copy(ones[:], ones_f32[:])

        def load_bias_row(src_1x512):
            t = bias_pool.tile([1, NBS], bf16, tag="br", name="brow")
            nc.gpsimd.dma_start(t[:], src_1x512)
            return t

    # --- helpers -----------------------------------------------------------
        def transpose_block(src_128x128_f32, dst_bf16_slice):
            ps = tpsum.tile([P, P], f32, tag="tps")
            nc.tensor.transpose(ps[:], src_128x128_f32, ident[:])
            nc.scalar.copy(dst_bf16_slice, ps[:])

        def load_transposed(src_dram_2d):
            """[BC, 1024] DRAM f32 -> list of KT stationary tiles [128, BC] bf16."""
            kt_tiles = [
                statT.tile([P, BC], bf16, tag="sT", name=f"sT{k}") for k in range(KT)
            ]
            for m in range(MT):
                ld = ld_pool.tile([P, IN], f32, tag="ld")
                nc.sync.dma_start(ld[:], src_dram_2d[m * P : (m + 1) * P, :])
                for k in range(KT):
                    transpose_block(
                        ld[:, k * P : (k + 1) * P], kt_tiles[k][:, m * P : (m + 1) * P]
                    )
            return kt_tiles

        def load_weight_block(w_dram_2d, n):
            """[1024, 4H] f32 DRAM, gate-column block n -> bf16 tile (cast in DMA)."""
            t = wpool.tile([P, KT, NBS], bf16, tag="w")
            src = w_dram_2d.rearrange("(kt p) n -> p kt n", p=P)
            nc.gpsimd.dma_start(t[:], src[:, :, n * NBS : (n + 1) * NBS])
            return t

    # --- phase helpers -----------------------------------------------------
        def phase_A(l, n, hpT_list):
            """h_prev @ Wh part: 8 k-matmuls per m into fresh PSUM groups.
            Independent of the previous layer's h -> keeps PE fed (and the
            HAM clock warm) across half-layer boundaries."""
            wh_t = load_weight_block(wh_d[l], n)
            ps_list = []
            for m in range(MT):
                ps = gpsum.tile([P, NBS], f32, tag="gps", name=f"ps{l}_{n}_{m}")
                for k in range(KT):
                    nc.tensor.matmul(
                        ps[:],
                        hpT_list[k][:, m * P : (m + 1) * P],
                        wh_t[:, k, :],
                        start=(k == 0),
                        stop=False,
                    )
                ps_list.append(ps)
            return ps_list

        def phase_B(l, n, q, xT_list, ps_list):
            """x @ Wi part + rank-1 bias matmul + activated PSUM evacuation."""
            wi_t = load_weight_block(wi_d[l], n)
            brow = load_bias_row(bias_d[l : l + 1, n * NBS : (n + 1) * NBS])
            out = []
            for m in range(MT):
                ps = ps_list[m]
                for k in range(KT):
                    nc.tensor.matmul(
                        ps[:],
                        xT_list[k][:, m * P : (m + 1) * P],
                        wi_t[:, k, :],
                        start=False,
                        stop=False,
                    )
                nc.tensor.matmul(ps[:], ones[:], brow[:], start=False, stop=True)
                g = gates_pool.tile([P, NBS], f32, tag="g", name=f"g{l}_{n}_{m}")
                nc.scalar.activation(g[:], ps[:], Tanh if q == 2 else Sigmoid)
                out.append(g)
            return out

        def fc_phase_A(n, h3T):
            """First-half fc matmuls (k 0..3) — hoistable across the last
            layer boundary since k 0..3 of h3^T are ready after hf=0."""
            wf_t = load_weight_block(fcw_d, n)
            ps_list = []
            for m in range(MT):
                ps = gpsum.tile([P, NBS], f32, tag="gps", name=f"psfc{n}_{m}")
                for k in range(KT // 2):
                    nc.tensor.matmul(
                        ps[:],
                        h3T[k][:, m * P : (m + 1) * P],
                        wf_t[:, k, :],
                        start=(k == 0),
                        stop=False,
                    )
                ps_list.append(ps)
            return ps_list, wf_t

        def emit_updates(l, hf, gate_sb, xT_next):
            for m in range(MT):
                i_t = gate_sb[(0, m)]
                f_t = gate_sb[(1, m)]
                g_t = gate_sb[(2, m)]
                o_t = gate_sb[(3, m)]
                cp = cwork.tile([P, NBS], f32, tag="cw", name="cp")
                nc.sync.dma_start(
                    cp[:],
                    cp_d[l][m * P : (m + 1) * P, hf * NBS : (hf + 1) * NBS],
                )
                nc.vector.tensor_mul(f_t[:], f_t[:], cp[:])  # f*c
                nc.vector.tensor_mul(i_t[:], i_t[:], g_t[:])  # i*g
                cnew = cwork.tile([P, NBS], f32, tag="cw", name="cnew")
                nc.vector.tensor_add(cnew[:], f_t[:], i_t[:])
                nc.sync.dma_start(
                    co_d[l][m * P : (m + 1) * P, hf * NBS : (hf + 1) * NBS],
                    cnew[:],
                )
                th = cwork.tile([P, NBS], f32, tag="cw", name="th")
                nc.scalar.activation(th[:], cnew[:], Tanh)
                h_t = cwork.tile([P, NBS], f32, tag="cw", name="h_t")
                nc.vector.tensor_mul(h_t[:], o_t[:], th[:])
                nc.sync.dma_start(
                    ho_d[l][m * P : (m + 1) * P, hf * NBS : (hf + 1) * NBS],
                    h_t[:],
                )
                # transpose h half-block into next layer's stationary tiles
                for kk in range(NBS // P):
                    kg = hf * (NBS // P) + kk
                    transpose_block(
                        h_t[:, kk * P : (kk + 1) * P],
                        xT_next[kg][:, m * P : (m + 1) * P],
                    )

        def prep_hprevT_mblock(l, hpT_list, m):
            ld = ld_pool.tile([P, IN], f32, tag="ld", name="hpld")
            nc.sync.dma_start(ld[:], hp_d[l][m * P : (m + 1) * P, :])
            for k in range(KT):
                transpose_block(
                    ld[:, k * P : (k + 1) * P], hpT_list[k][:, m * P : (m + 1) * P]
                )

    # --- prologue: x^T and h_prev[0]^T -------------------------------------
        xT = load_transposed(x_d)
        hpT = load_transposed(hp_d[0])

    # --- layers ------------------------------------------------------------
        pending_A = None
        for l in range(L):
            xT_next = [
                statT.tile([P, BC], bf16, tag="sT", name=f"xTn{l}_{k}")
                for k in range(KT)
            ]
            hpT_next = (
                [
                    statT.tile([P, BC], bf16, tag="sT", name=f"hpTn{l}_{k}")
                    for k in range(KT)
                ]
                if l + 1 < L
                else None
            )
            for hf in range(HF):
                n_list = [2 * q + hf for q in range(4)]
                gate_sb = {}
                for q, n in enumerate(n_list):
                    if q == 0 and pending_A is not None:
                        ps_list = pending_A
                        pending_A = None
                    else:
                        ps_list = phase_A(l, n, hpT)
                    gs = phase_B(l, n, q, xT, ps_list)
                    for m in range(MT):
                        gate_sb[(q, m)] = gs[m]
                    # spread next layer's h_prev^T prep across hf=0 n-blocks
                    if hf == 0 and hpT_next is not None:
                        prep_hprevT_mblock(l + 1, hpT_next, q)
                # hoist the next half's h_prev matmuls ahead of the
                # elementwise epilogue so PE never idles at the boundary
                if hf == 0:
                    pending_A = phase_A(l, 1, hpT)
                elif l + 1 < L:
                    pending_A = phase_A(l + 1, 0, hpT_next)
                elif l == L - 1:
                    # last boundary: hoist the first-half fc matmuls (k 0..3
                    # use h3^T tiles produced by the hf=0 updates)
                    pending_A = fc_phase_A(0, xT_next)
                emit_updates(l, hf, gate_sb, xT_next)

            xT = xT_next
            hpT = hpT_next

    # --- final fc: y = h3 @ fc_W + fc_b ------------------------------------
        for n in range(FC_NB):
            if n == 0 and pending_A is not None:
                ps_list, wf_t = pending_A
                pending_A = None
            else:
                ps_list, wf_t = fc_phase_A(n, xT)
            brow = load_bias_row(fcb_d[:, n * NBS : (n + 1) * NBS])
            for m in range(MT):
                ps = ps_list[m]
                for k in range(KT // 2, KT):
                    nc.tensor.matmul(
                        ps[:],
                        xT[k][:, m * P : (m + 1) * P],
                        wf_t[:, k, :],
                        start=False,
                        stop=False,
                    )
                nc.tensor.matmul(ps[:], ones[:], brow[:], start=False, stop=True)
                y_sb = gates_pool.tile([P, NBS], f32, tag="g", name=f"y{n}_{m}")
                nc.scalar.copy(y_sb[:], ps[:])
                nc.sync.dma_start(
                    y_d[m * P : (m + 1) * P, n * NBS : (n + 1) * NBS], y_sb[:]
                )


def _build_nc():
    nc = bacc.Bacc("TRN2", target_bir_lowering=False, debug=False, num_devices=NCORES)
    _emit(nc)
    nc.compile()
    return nc


def kernel(x, h_prev, c_prev, Wi, Wh, bi, bh, fc_W, fc_b, **extra):
    nc = _cache.get("nc")
    if nc is None:
        nc = _cache["nc"] = _build_nc()

    bias = np.ascontiguousarray((np.asarray(bi) + np.asarray(bh)).astype(np.float32))
    wi = np.ascontiguousarray(np.asarray(Wi, dtype=np.float32))
    wh = np.ascontiguousarray(np.asarray(Wh, dtype=np.float32))
    fcw = np.ascontiguousarray(np.asarray(fc_W, dtype=np.float32))
    fcb = np.ascontiguousarray(np.asarray(fc_b, dtype=np.float32).reshape(1, OUT))
    x = np.asarray(x, dtype=np.float32)
    h_prev = np.asarray(h_prev, dtype=np.float32)
    c_prev = np.asarray(c_prev, dtype=np.float32)

    in_maps = []
    for i in range(NCORES):
        b0 = i * BC
        in_maps.append(
            {
                "x": np.ascontiguousarray(x[b0 : b0 + BC]),
                "h_prev": np.ascontiguousarray(h_prev[:, b0 : b0 + BC]),
                "c_prev": np.ascontiguousarray(c_prev[:, b0 : b0 + BC]),
                "Wi": wi,
                "Wh": wh,
                "bias": bias,
                "fc_W": fcw,
                "fc_b": fcb,
            }
        )

    res = run_bass_kernel_spmd(nc, in_maps, list(range(NCORES)))
    _cache["last_result"] = res
    y = np.concatenate([res.results[i]["y"] for i in range(NCORES)], axis=0)
    h_out = np.concatenate([res.results[i]["h_out"] for i in range(NCORES)], axis=1)
    c_out = np.concatenate([res.results[i]["c_out"] for i in range(NCORES)], axis=1)
    return y, h_out, c_out
